# revision 8
# baseline (speedup 1.0000x reference)
"""BiDAF forward pass as a Bass/Tile kernel for Trainium2, data-parallel over 8 cores.

Activations are feature-major: [feat_rows, T, nb] SBUF tiles (batch column per
example). LSTM scans run fwd+bwd as two interleaved dependency chains, gate
matmuls accumulating onto PSUM pre-loaded with the input projections.
"""
import sys
sys.path.insert(0, "/opt/trn_rl_repo")
import numpy as np
from contextlib import ExitStack

import concourse.bass as bass
import concourse.bacc as bacc
import concourse.tile as tile
import concourse.mybir as mybir
from concourse.masks import make_identity

F32 = mybir.dt.float32
I32 = mybir.dt.int32
AF = mybir.ActivationFunctionType
OP = mybir.AluOpType

H = 100
D2 = 200
NBC = 4            # batch per core
CLEN, QLEN, WLEN = 512, 48, 16
QPAD = 64          # padded q stride: 4 ex * 64 = 256 tokens = 2 gather tiles
CHAR_V, CHAR_D, CC, CW = 262, 8, 100, 5
NPOS = WLEN - CW + 1  # 12 conv positions
WORD_V, WD = 50000, 100
T = CLEN
SEG = 32
P = 128


def rev_dim(a, dim):
    ap_list = [list(x) for x in a.ap]
    step, cnt = ap_list[dim]
    off = a.offset + step * (cnt - 1)
    ap_list[dim] = [-step, cnt]
    return bass.AP(tensor=a.tensor, offset=off, ap=ap_list)


def emit_bilstm(nc, sbuf, psum, xin_tiles, Wih_sb, Whh_sb, H_out, nb, name,
                reset_step=None):
    """BiLSTM, static unroll, 2 chains (fwd d=0 / bwd d=1), phase-grouped emission.

    xin_tiles: [(tile, rows)] K-chunks [rows, T, >=nb]; rows==101 tiles carry a
    ones row (bias) at row 100. Wih_sb[d][ci]: lhsT [rows, 400] (gates i,f,o,2g).
    Whh_sb[d]: [100, 400]. H_out[d]: [>=100, T, nb]; fwd h at pos t, bwd at
    pos T-1-t. reset_step: at this step, recurrent input for batch cols nb//2..
    is forced to zero state (fresh sequence start for q packed into ctx scan).
    """
    nseg = T // SEG
    nh = nb // 2
    zeros = sbuf.tile([H, nb], F32, name=f"zr{name}", tag=f"zr{name}")
    nc.vector.memset(zeros[:], 0.0)
    S = [[sbuf.tile([H, 6, nb], F32, name=f"S{name}{d}_{k}", tag=f"S{name}{d}_{k}")
          for k in range(2)] for d in range(2)]
    M = [sbuf.tile([H, 2, nb], F32, name=f"M{name}{d}", tag=f"M{name}{d}") for d in range(2)]
    U = [sbuf.tile([H, nb], F32, name=f"U{name}{d}", tag=f"U{name}{d}") for d in range(2)]
    TH = [sbuf.tile([H, nb], F32, name=f"TH{name}{d}", tag=f"TH{name}{d}") for d in range(2)]
    for d in range(2):
        nc.vector.memset(S[d][0][:], 0.0)
        nc.vector.memset(S[d][1][:], 0.0)
    gps = [[psum.tile([H, 4, SEG, nb], F32, space="PSUM", name=f"g{name}{d}_{r}",
                      tag=f"g{name}{d}_{r}") for r in range(2)] for d in range(2)]
    gc_per_bank = max(1, 512 // (SEG * nb))  # gate-chunks sharing one PSUM bank

    def precompute_seg(d, s):
        ring = s % 2
        t0 = s * SEG
        for gc in range(4):
            first = (gc % gc_per_bank) == 0  # start=True clears a whole bank
            for ci, (xt, rows) in enumerate(xin_tiles):
                if d == 0:
                    rhs = xt[:rows, t0:t0 + SEG, 0:nb]
                else:
                    rhs = rev_dim(xt[:rows, T - t0 - SEG:T - t0, 0:nb], 1)
                nc.tensor.matmul(
                    out=gps[d][ring][:, gc, :, :],
                    lhsT=Wih_sb[d][ci][:rows, bass.ts(gc, H)],
                    rhs=rhs, start=first, stop=False, skip_group_check=True)
                first = False

    for d in range(2):
        precompute_seg(d, 0)

    for t in range(T):
        s, off = divmod(t, SEG)
        ring = s % 2
        k, k1 = t % 2, (t + 1) % 2
        if off == 0 and s + 1 < nseg:
            for d in range(2):
                precompute_seg(d, s + 1)
        for d in range(2):
            if t == 0:
                hprev = zeros[:]
            else:
                pos_prev = (t - 1) if d == 0 else T - t
                hprev = H_out[d][:H, pos_prev, :]
            if reset_step is not None and t == reset_step and t > 0:
                for gc in range(4):
                    nc.tensor.matmul(out=gps[d][ring][:, gc, off, 0:nh],
                                     lhsT=Whh_sb[d][:, bass.ts(gc, H)],
                                     rhs=hprev[:, 0:nh], start=False, stop=True,
                                     skip_group_check=True)
                    nc.tensor.matmul(out=gps[d][ring][:, gc, off, nh:nb],
                                     lhsT=Whh_sb[d][:, bass.ts(gc, H)],
                                     rhs=zeros[:, nh:nb], start=False, stop=True,
                                     skip_group_check=True)
            else:
                for gc in range(4):
                    nc.tensor.matmul(out=gps[d][ring][:, gc, off, :],
                                     lhsT=Whh_sb[d][:, bass.ts(gc, H)],
                                     rhs=hprev, start=False, stop=True,
                                     skip_group_check=True)
        for d in range(2):
            nc.scalar.activation(out=S[d][k][:, 0:4, :], in_=gps[d][ring][:, :, off, :],
                                 func=AF.Sigmoid)
        if reset_step is not None and t == reset_step:
            for d in range(2):
                nc.vector.memset(S[d][k][:, 4, nh:], 0.0)
        for d in range(2):
            nc.vector.tensor_tensor(out=M[d][:], in0=S[d][k][:, 0:2, :],
                                    in1=S[d][k][:, 3:5, :], op=OP.mult)
            nc.vector.scalar_tensor_tensor(out=U[d][:], in0=M[d][:, 0, :], scalar=2.0,
                                           in1=S[d][k][:, 0, :], op0=OP.mult,
                                           op1=OP.subtract)
            nc.vector.tensor_tensor(out=S[d][k1][:, 4, :], in0=U[d][:], in1=M[d][:, 1, :],
                                    op=OP.add)
        for d in range(2):
            nc.scalar.activation(out=TH[d][:], in_=S[d][k1][:, 4, :], func=AF.Tanh)
        for d in range(2):
            pos = t if d == 0 else T - 1 - t
            nc.vector.tensor_tensor(out=H_out[d][:H, pos, :], in0=S[d][k][:, 2, :],
                                    in1=TH[d][:, :], op=OP.mult)


def build_kernel():
    nc = bacc.Bacc("TRN2", target_bir_lowering=False, debug=False)
    cchar = nc.dram_tensor("cchar", [NBC * CLEN, WLEN], I32, kind="ExternalInput")
    qchar = nc.dram_tensor("qchar", [NBC * QPAD, WLEN], I32, kind="ExternalInput")
    cword = nc.dram_tensor("cword", [NBC * CLEN, 1], I32, kind="ExternalInput")
    qword = nc.dram_tensor("qword", [NBC * QPAD, 1], I32, kind="ExternalInput")
    cemb = nc.dram_tensor("cemb", [CHAR_V, CHAR_D], F32, kind="ExternalInput")
    wemb = nc.dram_tensor("wemb", [WORD_V, WD], F32, kind="ExternalInput")
    convw = nc.dram_tensor("convw", [NPOS, P, CC], F32, kind="ExternalInput")
    convb = nc.dram_tensor("convb", [CC, 1], F32, kind="ExternalInput")
    hww = nc.dram_tensor("hww", [2, 2, 2, 2, 101, 100], F32, kind="ExternalInput")
    wih_ctx = nc.dram_tensor("wih_ctx", [2, 2, 101, 4 * H], F32, kind="ExternalInput")
    wih_m1 = nc.dram_tensor("wih_m1", [2, 8, 101, 4 * H], F32, kind="ExternalInput")
    wih_m2 = nc.dram_tensor("wih_m2", [2, 2, 101, 4 * H], F32, kind="ExternalInput")
    wih_out = nc.dram_tensor("wih_out", [2, 2, 101, 4 * H], F32, kind="ExternalInput")
    whh_all = nc.dram_tensor("whh_all", [4, 2, H, 4 * H], F32, kind="ExternalInput")
    attv = nc.dram_tensor("attv", [6, H, 1], F32, kind="ExternalInput")
    headw = nc.dram_tensor("headw", [2, 10, 101, 1], F32, kind="ExternalInput")
    onesd = nc.dram_tensor("onesd", [1, 4096], F32, kind="ExternalInput")
    out_d = nc.dram_tensor("out", [2, NBC, CLEN], F32, kind="ExternalOutput")

    with ExitStack() as ctx:
        tc = ctx.enter_context(tile.TileContext(nc))
        cons = ctx.enter_context(tc.tile_pool(name="cons", bufs=1))
        sbuf = ctx.enter_context(tc.tile_pool(name="sbuf", bufs=2))
        sb3 = ctx.enter_context(tc.tile_pool(name="sb3", bufs=3))

        ident = cons.tile([P, P], F32, name="ident")
        make_identity(nc, ident[:])
        onesc = cons.tile([1, P], F32, name="onesc")
        nc.sync.dma_start(out=onesc[:], in_=onesd[:, :P])
        onescol = cons.tile([P, 1], F32, name="onescol")
        nc.vector.memset(onescol[:], 1.0)

        convw_sb = cons.tile([P, NPOS, CC], F32, name="convw_sb")
        nc.sync.dma_start(out=convw_sb[:], in_=convw[:].rearrange("a p c -> p a c"))
        convb_sb = cons.tile([CC, 1], F32, name="convb_sb")
        nc.sync.dma_start(out=convb_sb[:], in_=convb[:])
        hww_sb = cons.tile([101, 2, 2, 2, 2, 100], F32, name="hww_sb")
        nc.sync.dma_start(out=hww_sb[:], in_=hww[:].rearrange("l g o k r c -> r l g o k c"))
        attv_sb = cons.tile([H, 6], F32, name="attv_sb")
        nc.sync.dma_start(out=attv_sb[:], in_=attv[:].rearrange("v h o -> h (v o)"))
        headw_sb = cons.tile([101, 2, 10], F32, name="headw_sb")
        nc.sync.dma_start(out=headw_sb[:], in_=headw[:].rearrange("e c r o -> r e (c o)"))

        def load_wih(pool, dram, nchunks, lname):
            out = []
            for d in range(2):
                tiles = []
                for ci in range(nchunks):
                    wt = pool.tile([101, 4 * H], F32, name=f"wi_{lname}_{d}_{ci}",
                                   tag=f"wi_{lname}_{d}_{ci}")
                    nc.sync.dma_start(out=wt[:], in_=dram[d, ci, :, :])
                    tiles.append(wt)
                out.append(tiles)
            return out

        def load_whh(pool, li, lname):
            pair = []
            for d in range(2):
                wt = pool.tile([H, 4 * H], F32, name=f"wh_{lname}_{d}", tag=f"wh_{lname}_{d}")
                nc.sync.dma_start(out=wt[:], in_=whh_all[li, d, :, :])
                pair.append(wt)
            return pair

        wih_sb = {"ctx": load_wih(cons, wih_ctx, 2, "ctx")}
        whh_sb = {"ctx": load_whh(cons, 0, "ctx")}

        # ---- phase A: embeddings + char conv ----
        pearly_cm = tc.tile_pool(name="pearly", bufs=1)
        pearly = pearly_cm.__enter__()
        xc0 = pearly.tile([H + 1, T, 8], F32, name="xc0")
        xc1 = pearly.tile([H, T, 8], F32, name="xc1")
        nc.sync.dma_start(out=xc0[H:H + 1, :, :],
                          in_=onesd[:].rearrange("o (t n) -> o t n", t=T))
        nc.vector.memset(xc0[:H, :, 4:], 0.0)
        nc.vector.memset(xc1[:, :, 4:], 0.0)

        with tc.tile_pool(name="psA", bufs=1, space="PSUM") as psA:
            def emb_tile(idx_dram, widx_dram, row0, out0, out1, src0, src1, tag):
                """One 128-token gather+conv tile. out0/out1: dest APs for char
                conv/word features; src0/src1: matching source views of the
                [100,128] results (token selectors)."""
                ix = sb3.tile([P, WLEN], I32, name=f"ix{tag}", tag="ix")
                nc.sync.dma_start(out=ix[:], in_=idx_dram[row0:row0 + P, :])
                wx = sb3.tile([P, 1], I32, name=f"wx{tag}", tag="wx")
                nc.sync.dma_start(out=wx[:], in_=widx_dram[row0:row0 + P, :])
                E = sb3.tile([P, WLEN, CHAR_D], F32, name=f"E{tag}", tag="E")
                for j in range(WLEN):
                    nc.gpsimd.indirect_dma_start(
                        out=E[:, j, :], out_offset=None, in_=cemb[:],
                        in_offset=bass.IndirectOffsetOnAxis(ap=ix[:, j:j + 1], axis=0))
                W = sb3.tile([P, WD], F32, name=f"W{tag}", tag="W")
                nc.gpsimd.indirect_dma_start(
                    out=W[:], out_offset=None, in_=wemb[:],
                    in_offset=bass.IndirectOffsetOnAxis(ap=wx[:, :1], axis=0))
                Et_ps = psA.tile([P, P], F32, space="PSUM", name=f"Etp{tag}", tag="Etp")
                nc.tensor.transpose(out=Et_ps[:], in_=E[:].rearrange("p a b -> p (a b)"),
                                    identity=ident[:])
                Et = sb3.tile([P, P], F32, name=f"Et{tag}", tag="Et")
                nc.vector.tensor_copy(out=Et[:], in_=Et_ps[:])
                Wt_ps = psA.tile([WD, P], F32, space="PSUM", name=f"Wtp{tag}", tag="Wtp")
                nc.tensor.transpose(out=Wt_ps[:], in_=W[:], identity=ident[:])
                nc.vector.tensor_copy(out=out1, in_=src1(Wt_ps))
                cv = psA.tile([CC, NPOS, P], F32, space="PSUM", name=f"cv{tag}", tag="cv")
                for tt in range(NPOS):
                    nc.tensor.matmul(out=cv[:, tt, :], lhsT=convw_sb[:, tt, :], rhs=Et[:],
                                     start=(tt % 4 == 0), stop=True, skip_group_check=True)
                mx = sb3.tile([CC, P], F32, name=f"mx{tag}", tag="mx")
                cv_view = bass.AP(tensor=cv[:].tensor, offset=cv[:].offset,
                                  ap=[list(cv[:].ap[0]), [1, P], [P, NPOS]])
                nc.vector.tensor_reduce(out=mx[:], in_=cv_view,
                                        axis=mybir.AxisListType.X, op=OP.max)
                nc.scalar.activation(out=out0, in_=src0(mx), func=AF.Relu,
                                     bias=convb_sb[:])

            def csel(t):  # full 128-token selector
                return lambda tl: tl[:CC, :] if t == 0 else tl[:WD, :]

            for w in range(16):
                b, t4 = divmod(w, 4)
                emb_tile(cchar, cword, 128 * w,
                         xc0[0:CC, t4 * 128:(t4 + 1) * 128, b],
                         xc1[0:WD, t4 * 128:(t4 + 1) * 128, b],
                         csel(0), csel(1), f"c{w}")
            for w in range(2):
                def qdst(xt, rows):
                    base = xt[0:rows, 0:1, 4 + 2 * w]
                    return bass.AP(tensor=base.tensor, offset=base.offset,
                                   ap=[list(base.ap[0]), [1, 2], [8, QLEN]])

                def qsrc(rows):
                    def f(tl):
                        a = tl[:rows, :]
                        return bass.AP(tensor=a.tensor, offset=a.offset,
                                       ap=[list(a.ap[0]), [QPAD, 2], [1, QLEN]])
                    return f
                emb_tile(qchar, qword, 128 * w, qdst(xc0, CC), qdst(xc1, WD),
                         qsrc(CC), qsrc(WD), f"q{w}")

        # ---- phase A2: highway ----
        xs = [xc0, xc1]
        with tc.tile_pool(name="psH", bufs=2, space="PSUM") as psH:
            nx0 = pearly.tile([H + 1, T, 8], F32, name="nx0")
            nx1 = pearly.tile([H, T, 8], F32, name="nx1")
            nc.sync.dma_start(out=nx0[H:H + 1, :, :],
                              in_=onesd[:].rearrange("o (t n) -> o t n", t=T))
            for layer in range(2):
                for nch in range(8):
                    t0 = nch * 64
                    for oc in range(2):
                        hp = psH.tile([H, 64, 8], F32, space="PSUM",
                                      name=f"hp{layer}{nch}{oc}", tag="hp")
                        gp = psH.tile([H, 64, 8], F32, space="PSUM",
                                      name=f"gp{layer}{nch}{oc}", tag="gp")
                        for kc in range(2):
                            rows = 101 if kc == 0 else 100
                            rhs = xs[kc][:rows, t0:t0 + 64, :]
                            nc.tensor.matmul(out=hp[:], lhsT=hww_sb[:rows, layer, 0, oc, kc, :],
                                             rhs=rhs, start=(kc == 0), stop=(kc == 1))
                            nc.tensor.matmul(out=gp[:], lhsT=hww_sb[:rows, layer, 1, oc, kc, :],
                                             rhs=rhs, start=(kc == 0), stop=(kc == 1))
                        hs = sb3.tile([H, 64, 8], F32, name=f"hs{layer}{nch}{oc}", tag="hs")
                        gs = sb3.tile([H, 64, 8], F32, name=f"gs{layer}{nch}{oc}", tag="gs")
                        nc.scalar.activation(out=hs[:], in_=hp[:], func=AF.Relu)
                        nc.scalar.activation(out=gs[:], in_=gp[:], func=AF.Sigmoid)
                        xo = xs[oc][0:H, t0:t0 + 64, :]
                        nxs = [nx0, nx1] if layer == 0 else [xc0, xc1]
                        dst = nxs[oc][0:H, t0:t0 + 64, :]
                        nc.vector.tensor_tensor(out=hs[:], in0=hs[:], in1=xo, op=OP.subtract)
                        nc.vector.tensor_tensor(out=hs[:], in0=gs[:], in1=hs[:], op=OP.mult)
                        nc.vector.tensor_tensor(out=dst, in0=hs[:], in1=xo, op=OP.add)
                xs = [nx0, nx1] if layer == 0 else [xc0, xc1]
        xf0, xf1 = xs

        # ---- phase B: ctx scan ----
        Hf = cons.tile([H + 1, T, 8], F32, name="Hf")
        Hb = cons.tile([H, T, 8], F32, name="Hb")
        nc.sync.dma_start(out=Hf[H:H + 1, :, :],
                          in_=onesd[:].rearrange("o (t n) -> o t n", t=T))
        with tc.tile_pool(name="psB", bufs=1, space="PSUM") as psB:
            emit_bilstm(nc, sbuf, psB, [(xf0, H + 1), (xf1, H)],
                        wih_sb["ctx"], whh_sb["ctx"], [Hf, Hb], 8, "ctx",
                        reset_step=T - QLEN)

        pearly_cm.__exit__(None, None, None)
        plate = ctx.enter_context(tc.tile_pool(name="plate", bufs=1))
        # ---- phase C: attention ----
        c2qf = plate.tile([H, T, NBC], F32, name="c2qf")
        c2qb = plate.tile([H, T, NBC], F32, name="c2qb")
        ccf = plate.tile([H, T, NBC], F32, name="ccf")
        ccb = plate.tile([H, T, NBC], F32, name="ccb")
        g4f = plate.tile([H, T, NBC], F32, name="g4f")
        g4b = plate.tile([H, T, NBC], F32, name="g4b")
        with tc.tile_pool(name="psC", bufs=2, space="PSUM") as psC, \
             tc.tile_pool(name="sbC", bufs=2) as sbC:
            for b in range(NBC):
                cwf = sbC.tile([H, T], F32, name=f"cwf{b}", tag="cwf")
                cwb = sbC.tile([H, T], F32, name=f"cwb{b}", tag="cwb")
                nc.vector.tensor_scalar_mul(cwf[:], Hf[0:H, :, b], attv_sb[:, 0:1])
                nc.vector.tensor_scalar_mul(cwb[:], Hb[0:H, :, b], attv_sb[:, 1:2])
                qlp = psC.tile([1, QLEN], F32, space="PSUM", name=f"qlp{b}", tag="psml")
                nc.tensor.matmul(out=qlp[:], lhsT=attv_sb[:, 2:3],
                                 rhs=Hf[0:H, 0:QLEN, 4 + b], start=True, stop=False)
                nc.tensor.matmul(out=qlp[:], lhsT=attv_sb[:, 3:4],
                                 rhs=Hb[0:H, 0:QLEN, 4 + b], start=False, stop=True)
                qls = sbC.tile([1, QLEN], F32, name=f"qls{b}", tag="qls")
                nc.vector.tensor_copy(out=qls[:], in_=qlp[:])
                qt_ps = psC.tile([QLEN, H, 2], F32, space="PSUM", name=f"qt{b}", tag="pbig")
                nc.tensor.transpose(out=qt_ps[:, :, 0], in_=Hf[0:H, 0:QLEN, 4 + b],
                                    identity=ident[:H, :H])
                nc.tensor.transpose(out=qt_ps[:, :, 1], in_=Hb[0:H, 0:QLEN, 4 + b],
                                    identity=ident[:H, :H])
                qt = sbC.tile([QLEN, H, 2], F32, name=f"qts{b}", tag="qts")
                nc.vector.tensor_copy(out=qt[:], in_=qt_ps[:])
                vsb = sbC.tile([P, 4], F32, name=f"vsb{b}", tag="vsb")
                ctm = sbC.tile([P, D2, 4], F32, name=f"ctm{b}", tag="ctm")
                for ic in range(4):
                    isl = slice(ic * P, (ic + 1) * P)
                    sp = psC.tile([P, QLEN], F32, space="PSUM", name=f"sp{b}{ic}", tag="pbig")
                    nc.tensor.matmul(out=sp[:], lhsT=cwf[:, isl],
                                     rhs=Hf[0:H, 0:QLEN, 4 + b], start=True, stop=False)
                    nc.tensor.matmul(out=sp[:], lhsT=cwb[:, isl],
                                     rhs=Hb[0:H, 0:QLEN, 4 + b], start=False, stop=False)
                    nc.tensor.matmul(out=sp[:], lhsT=onesc[:], rhs=qls[:],
                                     start=False, stop=True)
                    rmax = sbC.tile([P, 1], F32, name=f"rmax{b}{ic}", tag="rmax")
                    nc.vector.tensor_reduce(out=rmax[:], in_=sp[:],
                                            axis=mybir.AxisListType.X, op=OP.max)
                    nmax = sbC.tile([P, 1], F32, name=f"nmax{b}{ic}", tag="nmax")
                    nc.vector.tensor_scalar_mul(nmax[:], rmax[:], -1.0)
                    esb = sbC.tile([P, QLEN], F32, name=f"esb{b}{ic}", tag="esb")
                    nc.scalar.activation(out=esb[:], in_=sp[:], func=AF.Exp, bias=nmax[:])
                    rsum = sbC.tile([P, 1], F32, name=f"rsum{b}{ic}", tag="rsum")
                    nc.vector.tensor_reduce(out=rsum[:], in_=esb[:],
                                            axis=mybir.AxisListType.X, op=OP.add)
                    rinv = sbC.tile([P, 1], F32, name=f"rinv{b}{ic}", tag="rinv")
                    nc.vector.reciprocal(out=rinv[:], in_=rsum[:])
                    nc.vector.tensor_scalar_mul(esb[:], esb[:], rinv[:])
                    at_ps = psC.tile([QLEN, P], F32, space="PSUM", name=f"at{b}{ic}", tag="pbig")
                    nc.tensor.transpose(out=at_ps[:], in_=esb[:], identity=ident[:])
                    ats = sbC.tile([QLEN, P], F32, name=f"ats{b}{ic}", tag="ats")
                    nc.vector.tensor_copy(out=ats[:], in_=at_ps[:])
                    c2p = psC.tile([H, P, 2], F32, space="PSUM", name=f"c2p{b}{ic}", tag="pbig")
                    nc.tensor.matmul(out=c2p[:, :, 0], lhsT=qt[:, :, 0], rhs=ats[:],
                                     start=True, stop=True)
                    nc.tensor.matmul(out=c2p[:, :, 1], lhsT=qt[:, :, 1], rhs=ats[:],
                                     start=True, stop=True)
                    nc.vector.tensor_copy(out=c2qf[:, isl, b], in_=c2p[:, :, 0])
                    nc.vector.tensor_copy(out=c2qb[:, isl, b], in_=c2p[:, :, 1])
                    vp = psC.tile([P, 1], F32, space="PSUM", name=f"vp{b}{ic}", tag="psml")
                    nc.tensor.matmul(out=vp[:], lhsT=Hf[0:H, isl, b], rhs=attv_sb[:, 4:5],
                                     start=True, stop=False)
                    nc.tensor.matmul(out=vp[:], lhsT=Hb[0:H, isl, b], rhs=attv_sb[:, 5:6],
                                     start=False, stop=True)
                    nc.vector.tensor_tensor(out=vsb[:, ic:ic + 1], in0=vp[:], in1=rmax[:],
                                            op=OP.add)
                    ctp = psC.tile([P, D2], F32, space="PSUM", name=f"ctp{b}{ic}", tag="pbig")
                    nc.tensor.transpose(out=ctp[:, 0:H], in_=Hf[0:H, isl, b],
                                        identity=ident[:H, :H])
                    nc.tensor.transpose(out=ctp[:, H:D2], in_=Hb[0:H, isl, b],
                                        identity=ident[:H, :H])
                    nc.vector.tensor_copy(out=ctm[:, :, ic], in_=ctp[:])
                m1 = sbC.tile([P, 1], F32, name=f"m1{b}", tag="m1")
                nc.vector.tensor_reduce(out=m1[:], in_=vsb[:],
                                        axis=mybir.AxisListType.X, op=OP.max)
                m1t_ps = psC.tile([1, P], F32, space="PSUM", name=f"m1t{b}", tag="psml")
                nc.tensor.transpose(out=m1t_ps[:], in_=m1[:], identity=ident[:])
                m1t = sbC.tile([1, P], F32, name=f"m1ts{b}", tag="m1ts")
                nc.vector.tensor_copy(out=m1t[:], in_=m1t_ps[:])
                gm = sbC.tile([1, 1], F32, name=f"gm{b}", tag="gm")
                nc.vector.tensor_reduce(out=gm[:], in_=m1t[:],
                                        axis=mybir.AxisListType.X, op=OP.max)
                nc.vector.tensor_scalar_mul(gm[:], gm[:], -1.0)
                gmb_ps = psC.tile([P, 1], F32, space="PSUM", name=f"gmb{b}", tag="psml")
                nc.tensor.matmul(out=gmb_ps[:], lhsT=onesc[:], rhs=gm[:], start=True, stop=True)
                gmb = sbC.tile([P, 1], F32, name=f"gmbs{b}", tag="gmbs")
                nc.vector.tensor_copy(out=gmb[:], in_=gmb_ps[:])
                ev = sbC.tile([P, 4], F32, name=f"ev{b}", tag="ev")
                nc.scalar.activation(out=ev[:], in_=vsb[:], func=AF.Exp, bias=gmb[:])
                es = sbC.tile([P, 1], F32, name=f"es{b}", tag="es")
                nc.vector.tensor_reduce(out=es[:], in_=ev[:],
                                        axis=mybir.AxisListType.X, op=OP.add)
                ssp = psC.tile([1, 1], F32, space="PSUM", name=f"ssp{b}", tag="psml")
                nc.tensor.matmul(out=ssp[:], lhsT=es[:], rhs=onescol[:], start=True, stop=True)
                ss = sbC.tile([1, 1], F32, name=f"ss{b}", tag="ss")
                nc.vector.tensor_copy(out=ss[:], in_=ssp[:])
                nc.vector.reciprocal(out=ss[:], in_=ss[:])
                sb_ps = psC.tile([P, 1], F32, space="PSUM", name=f"sbp{b}", tag="psml")
                nc.tensor.matmul(out=sb_ps[:], lhsT=onesc[:], rhs=ss[:], start=True, stop=True)
                sbv = sbC.tile([P, 1], F32, name=f"sbv{b}", tag="sbv")
                nc.vector.tensor_copy(out=sbv[:], in_=sb_ps[:])
                batt = sbC.tile([P, 4], F32, name=f"batt{b}", tag="batt")
                nc.vector.tensor_scalar_mul(batt[:], ev[:], sbv[:])
                q2p = psC.tile([1, D2], F32, space="PSUM", name=f"q2p{b}", tag="psml")
                for ic in range(4):
                    nc.tensor.matmul(out=q2p[:], lhsT=batt[:, ic:ic + 1], rhs=ctm[:, :, ic],
                                     start=(ic == 0), stop=(ic == 3))
                q2ps = sbC.tile([1, D2], F32, name=f"q2ps{b}", tag="q2psb")
                nc.vector.tensor_copy(out=q2ps[:], in_=q2p[:])
                q2tp = psC.tile([H, 2], F32, space="PSUM", name=f"q2tp{b}", tag="psml")
                nc.tensor.transpose(out=q2tp[:, 0:1], in_=q2ps[:, 0:H], identity=ident[:1, :1])
                nc.tensor.transpose(out=q2tp[:, 1:2], in_=q2ps[:, H:D2], identity=ident[:1, :1])
                q2s = sbC.tile([H, 2], F32, name=f"q2s{b}", tag="q2s")
                nc.vector.tensor_copy(out=q2s[:], in_=q2tp[:])
                nc.vector.tensor_tensor(out=ccf[:, :, b], in0=Hf[0:H, :, b],
                                        in1=c2qf[:, :, b], op=OP.mult)
                nc.vector.tensor_tensor(out=ccb[:, :, b], in0=Hb[0:H, :, b],
                                        in1=c2qb[:, :, b], op=OP.mult)
                nc.vector.tensor_scalar_mul(g4f[:, :, b], Hf[0:H, :, b], q2s[:, 0:1])
                nc.vector.tensor_scalar_mul(g4b[:, :, b], Hb[0:H, :, b], q2s[:, 1:2])

        # ---- phases D/E/F: mod1, mod2, out scans ----
        g_chunks = [(Hf, H + 1), (Hb, H), (c2qf, H), (c2qb, H),
                    (ccf, H), (ccb, H), (g4f, H), (g4b, H)]
        M1f = plate.tile([H + 1, T, NBC], F32, name="M1f")
        M1b = plate.tile([H, T, NBC], F32, name="M1b")
        nc.sync.dma_start(out=M1f[H:H + 1, :, :],
                          in_=onesd[:, :T * NBC].rearrange("o (t n) -> o t n", t=T))
        with tc.tile_pool(name="psD", bufs=1, space="PSUM") as psD, \
             tc.tile_pool(name="pw1", bufs=1) as pw1:
            emit_bilstm(nc, sbuf, psD, g_chunks, load_wih(pw1, wih_m1, 8, "m1"),
                        load_whh(pw1, 1, "m1"), [M1f, M1b], NBC, "m1")
        M2f = plate.tile([H + 1, T, NBC], F32, name="M2f")
        M2b = plate.tile([H, T, NBC], F32, name="M2b")
        nc.sync.dma_start(out=M2f[H:H + 1, :, :],
                          in_=onesd[:, :T * NBC].rearrange("o (t n) -> o t n", t=T))
        with tc.tile_pool(name="psE", bufs=1, space="PSUM") as psE, \
             tc.tile_pool(name="pw2", bufs=1) as pw2:
            emit_bilstm(nc, sbuf, psE, [(M1f, H + 1), (M1b, H)],
                        load_wih(pw2, wih_m2, 2, "m2"), load_whh(pw2, 2, "m2"),
                        [M2f, M2b], NBC, "m2")
        Of = plate.tile([H + 1, T, NBC], F32, name="Of")
        Ob = plate.tile([H, T, NBC], F32, name="Ob")
        nc.sync.dma_start(out=Of[H:H + 1, :, :],
                          in_=onesd[:, :T * NBC].rearrange("o (t n) -> o t n", t=T))
        with tc.tile_pool(name="psF", bufs=1, space="PSUM") as psF, \
             tc.tile_pool(name="pw3", bufs=1) as pw3:
            emit_bilstm(nc, sbuf, psF, [(M2f, H + 1), (M2b, H)],
                        load_wih(pw3, wih_out, 2, "out"), load_whh(pw3, 3, "out"),
                        [Of, Ob], NBC, "out")

        # ---- phase G: heads ----
        with tc.tile_pool(name="psG", bufs=2, space="PSUM") as psG:
            for e, mpair in ((0, (M2f, M2b)), (1, (Of, Ob))):
                chunks = g_chunks + [(mpair[0], H + 1), (mpair[1], H)]
                for b in range(NBC):
                    hp = psG.tile([1, T], F32, space="PSUM", name=f"hd{e}{b}", tag="hd")
                    for ci, (tl, rows) in enumerate(chunks):
                        nc.tensor.matmul(out=hp[:], lhsT=headw_sb[:rows, e, ci:ci + 1],
                                         rhs=tl[:rows, :, b], start=(ci == 0),
                                         stop=(ci == 9))
                    hsb = sbuf.tile([1, T], F32, name=f"hsb{e}{b}", tag="hsb")
                    nc.vector.tensor_copy(out=hsb[:], in_=hp[:])
                    nc.gpsimd.dma_start(out=out_d[e, b, :].unsqueeze(0), in_=hsb[:])

    nc.finalize()
    return nc


_NC_CACHE = None


def _prep_gates(Wih, Whh, b):
    def reord(W):
        i, f, g, o = np.split(W, 4, axis=0)
        return np.concatenate([i, f, o, 2.0 * g], axis=0)
    return reord(Wih), reord(Whh), reord(b[:, None])[:, 0]


def _pack_lstm(p, chunk_rows):
    nch = len(chunk_rows)
    wih = np.zeros((2, nch, 101, 4 * H), np.float32)
    whh = np.zeros((2, H, 4 * H), np.float32)
    for d, sfx in enumerate(["f", "b"]):
        wi, wh, bb = _prep_gates(np.asarray(p[f"Wih_{sfx}"], np.float32),
                                 np.asarray(p[f"Whh_{sfx}"], np.float32),
                                 np.asarray(p[f"b_{sfx}"], np.float32))
        for ci, (s0, s1) in enumerate(chunk_rows):
            wih[d, ci, :s1 - s0, :] = wi[:, s0:s1].T
        wih[d, 0, 100, :] = bb
        whh[d] = wh.T
    return wih, whh


def kernel(c_char, q_char, c_word, q_word, char_emb_W, char_conv_W, char_conv_b,
           word_emb_W, highway, ctx_lstm, mod_lstm1, mod_lstm2, out_lstm, attn, heads):
    global _NC_CACHE
    from concourse.bass_utils import run_bass_kernel_spmd

    c_char = np.asarray(c_char); q_char = np.asarray(q_char)
    c_word = np.asarray(c_word); q_word = np.asarray(q_word)

    convw = np.zeros((NPOS, P, CC), np.float32)
    cw = np.asarray(char_conv_W, np.float32)  # [100, 8, 5]
    for tt in range(NPOS):
        for kk in range(CW):
            for ee in range(CHAR_D):
                convw[tt, 8 * tt + 8 * kk + ee, :] = cw[:, ee, kk]
    convb = np.asarray(char_conv_b, np.float32).reshape(CC, 1)

    hww = np.zeros((2, 2, 2, 2, 101, 100), np.float32)
    for l in range(2):
        for gi, (Wk, bk) in enumerate((("Wl", "bl"), ("Wg", "bg"))):
            Wm = np.asarray(highway[Wk], np.float32)[l]
            bv = np.asarray(highway[bk], np.float32)[l]
            for oc in range(2):
                for kc in range(2):
                    hww[l, gi, oc, kc, :100, :] = \
                        Wm[oc * 100:(oc + 1) * 100, kc * 100:(kc + 1) * 100].T
                hww[l, gi, oc, 0, 100, :] = bv[oc * 100:(oc + 1) * 100]

    wih_ctx, whh_ctx = _pack_lstm(ctx_lstm, [(0, 100), (100, 200)])
    wih_m1, whh_m1 = _pack_lstm(mod_lstm1, [(i * 100, (i + 1) * 100) for i in range(8)])
    wih_m2, whh_m2 = _pack_lstm(mod_lstm2, [(0, 100), (100, 200)])
    wih_out, whh_out = _pack_lstm(out_lstm, [(0, 100), (100, 200)])
    whh_all = np.stack([whh_ctx, whh_m1, whh_m2, whh_out], 0)

    attv = np.stack([np.asarray(attn["w_cq"], np.float32)[0:100],
                     np.asarray(attn["w_cq"], np.float32)[100:200],
                     np.asarray(attn["w_q"], np.float32)[0:100],
                     np.asarray(attn["w_q"], np.float32)[100:200],
                     np.asarray(attn["w_c"], np.float32)[0:100],
                     np.asarray(attn["w_c"], np.float32)[100:200]], 0)[:, :, None]

    headw = np.zeros((2, 10, 101, 1), np.float32)
    for e, (Wg_, bg_, Wm_, bm_) in enumerate(
            ((heads["Wsg"], heads["bsg"], heads["Wsm"], heads["bsm"]),
             (heads["Weg"], heads["beg"], heads["Wem"], heads["bem"]))):
        Wg_ = np.asarray(Wg_, np.float32); Wm_ = np.asarray(Wm_, np.float32)
        for ci in range(8):
            headw[e, ci, :100, 0] = Wg_[ci * 100:(ci + 1) * 100]
        headw[e, 0, 100, 0] = float(np.asarray(bg_)) + float(np.asarray(bm_))
        headw[e, 8, :100, 0] = Wm_[0:100]
        headw[e, 9, :100, 0] = Wm_[100:200]

    if _NC_CACHE is None:
        _NC_CACHE = build_kernel()
    nc = _NC_CACHE

    shared = {
        "cemb": np.asarray(char_emb_W, np.float32),
        "wemb": np.asarray(word_emb_W, np.float32),
        "convw": convw, "convb": convb, "hww": hww,
        "wih_ctx": wih_ctx, "wih_m1": wih_m1, "wih_m2": wih_m2, "wih_out": wih_out,
        "whh_all": whh_all, "attv": attv.astype(np.float32), "headw": headw,
        "onesd": np.ones((1, 4096), np.float32),
    }
    in_maps = []
    for core in range(8):
        sl = slice(core * NBC, (core + 1) * NBC)
        qc = np.zeros((NBC, QPAD, WLEN), np.int64)
        qc[:, :QLEN, :] = q_char[sl]
        qw = np.zeros((NBC, QPAD), np.int64)
        qw[:, :QLEN] = q_word[sl]
        m = dict(shared)
        m["cchar"] = np.ascontiguousarray(c_char[sl].reshape(NBC * CLEN, WLEN)).astype(np.int32)
        m["qchar"] = qc.reshape(NBC * QPAD, WLEN).astype(np.int32)
        m["cword"] = np.ascontiguousarray(c_word[sl].reshape(NBC * CLEN, 1)).astype(np.int32)
        m["qword"] = qw.reshape(NBC * QPAD, 1).astype(np.int32)
        in_maps.append(m)

    res = run_bass_kernel_spmd(nc, in_maps, core_ids=list(range(8)))
    outs = [r["out"] for r in res.results]
    full = np.concatenate(outs, axis=1)
    return np.stack([full[0], full[1]], 0).astype(np.float32)


# revision 11
# speedup vs baseline: 1.0162x; 1.0162x over previous
"""BiDAF forward pass as a Bass/Tile kernel for Trainium2, data-parallel over 8 cores.

Activations are feature-major: [feat_rows, T, nb] SBUF tiles (batch column per
example). LSTM scans run fwd+bwd as two interleaved dependency chains, gate
matmuls accumulating onto PSUM pre-loaded with the input projections.
"""
import sys
sys.path.insert(0, "/opt/trn_rl_repo")
import numpy as np
from contextlib import ExitStack

import concourse.bass as bass
import concourse.bacc as bacc
import concourse.tile as tile
import concourse.mybir as mybir
from concourse.masks import make_identity

F32 = mybir.dt.float32
I32 = mybir.dt.int32
AF = mybir.ActivationFunctionType
OP = mybir.AluOpType

H = 100
D2 = 200
NBC = 4            # batch per core
CLEN, QLEN, WLEN = 512, 48, 16
QPAD = 64          # padded q stride: 4 ex * 64 = 256 tokens = 2 gather tiles
CHAR_V, CHAR_D, CC, CW = 262, 8, 100, 5
NPOS = WLEN - CW + 1  # 12 conv positions
WORD_V, WD = 50000, 100
T = CLEN
SEG = 32
P = 128


def rev_dim(a, dim):
    ap_list = [list(x) for x in a.ap]
    step, cnt = ap_list[dim]
    off = a.offset + step * (cnt - 1)
    ap_list[dim] = [-step, cnt]
    return bass.AP(tensor=a.tensor, offset=off, ap=ap_list)


def emit_bilstm(nc, sbuf, psum, xin_tiles, Wih_sb, Whh_sb, H_out, nb, name,
                reset_step=None, seg=None):
    """BiLSTM, static unroll, 2 chains (fwd d=0 / bwd d=1), phase-grouped emission.

    xin_tiles: [(tile, rows)] K-chunks [rows, T, >=nb]; rows==101 tiles carry a
    ones row (bias) at row 100. Wih_sb[d][ci]: lhsT [rows, 400] (gates i,f,o,2g).
    Whh_sb[d]: [100, 400]. H_out[d]: [>=100, T, nb]; fwd h at pos t, bwd at
    pos T-1-t. reset_step: at this step, recurrent input for batch cols nb//2..
    is forced to zero state (fresh sequence start for q packed into ctx scan).
    """
    seg = seg if seg is not None else SEG
    nseg = T // seg
    nh = nb // 2
    zeros = sbuf.tile([H, nb], F32, name=f"zr{name}", tag=f"zr{name}")
    nc.vector.memset(zeros[:], 0.0)
    S = [[sbuf.tile([H, 6, nb], F32, name=f"S{name}{d}_{k}", tag=f"S{name}{d}_{k}")
          for k in range(2)] for d in range(2)]
    M = [sbuf.tile([H, 2, nb], F32, name=f"M{name}{d}", tag=f"M{name}{d}") for d in range(2)]
    U = [sbuf.tile([H, nb], F32, name=f"U{name}{d}", tag=f"U{name}{d}") for d in range(2)]
    TH = [sbuf.tile([H, nb], F32, name=f"TH{name}{d}", tag=f"TH{name}{d}") for d in range(2)]
    for d in range(2):
        nc.vector.memset(S[d][0][:], 0.0)
        nc.vector.memset(S[d][1][:], 0.0)
    gps = [[psum.tile([H, 4, seg, nb], F32, space="PSUM", name=f"g{name}{d}_{r}",
                      tag=f"g{name}{d}_{r}") for r in range(2)] for d in range(2)]
    gc_per_bank = max(1, 512 // (seg * nb))  # gate-chunks sharing one PSUM bank

    def precompute_seg(d, s):
        ring = s % 2
        t0 = s * seg
        for gc in range(4):
            first = (gc % gc_per_bank) == 0  # start=True clears a whole bank
            for ci, (xt, rows) in enumerate(xin_tiles):
                if d == 0:
                    rhs = xt[:rows, t0:t0 + seg, 0:nb]
                else:
                    rhs = rev_dim(xt[:rows, T - t0 - seg:T - t0, 0:nb], 1)
                nc.tensor.matmul(
                    out=gps[d][ring][:, gc, :, :],
                    lhsT=Wih_sb[d][ci][:rows, bass.ts(gc, H)],
                    rhs=rhs, start=first, stop=False, skip_group_check=True)
                first = False

    for d in range(2):
        precompute_seg(d, 0)

    for t in range(T):
        s, off = divmod(t, seg)
        ring = s % 2
        k, k1 = t % 2, (t + 1) % 2
        if off == 0 and s + 1 < nseg:
            for d in range(2):
                precompute_seg(d, s + 1)
        for d in range(2):
            if t == 0:
                hprev = zeros[:]
            else:
                pos_prev = (t - 1) if d == 0 else T - t
                hprev = H_out[d][:H, pos_prev, :]
            if reset_step is not None and t == reset_step and t > 0:
                for gc in range(4):
                    nc.tensor.matmul(out=gps[d][ring][:, gc, off, 0:nh],
                                     lhsT=Whh_sb[d][:, bass.ts(gc, H)],
                                     rhs=hprev[:, 0:nh], start=False, stop=True,
                                     skip_group_check=True)
                    nc.tensor.matmul(out=gps[d][ring][:, gc, off, nh:nb],
                                     lhsT=Whh_sb[d][:, bass.ts(gc, H)],
                                     rhs=zeros[:, nh:nb], start=False, stop=True,
                                     skip_group_check=True)
            else:
                for gc in range(4):
                    nc.tensor.matmul(out=gps[d][ring][:, gc, off, :],
                                     lhsT=Whh_sb[d][:, bass.ts(gc, H)],
                                     rhs=hprev, start=False, stop=True,
                                     skip_group_check=True)
        for d in range(2):
            nc.scalar.activation(out=S[d][k][:, 0:4, :], in_=gps[d][ring][:, :, off, :],
                                 func=AF.Sigmoid)
        if reset_step is not None and t == reset_step:
            for d in range(2):
                nc.vector.memset(S[d][k][:, 4, nh:], 0.0)
        for d in range(2):
            nc.vector.tensor_tensor(out=M[d][:], in0=S[d][k][:, 0:2, :],
                                    in1=S[d][k][:, 3:5, :], op=OP.mult)
            nc.vector.scalar_tensor_tensor(out=U[d][:], in0=M[d][:, 0, :], scalar=2.0,
                                           in1=S[d][k][:, 0, :], op0=OP.mult,
                                           op1=OP.subtract)
            nc.vector.tensor_tensor(out=S[d][k1][:, 4, :], in0=U[d][:], in1=M[d][:, 1, :],
                                    op=OP.add)
        for d in range(2):
            nc.scalar.activation(out=TH[d][:], in_=S[d][k1][:, 4, :], func=AF.Tanh)
        for d in range(2):
            pos = t if d == 0 else T - 1 - t
            nc.vector.tensor_tensor(out=H_out[d][:H, pos, :], in0=S[d][k][:, 2, :],
                                    in1=TH[d][:, :], op=OP.mult)


def build_kernel():
    nc = bacc.Bacc("TRN2", target_bir_lowering=False, debug=False)
    cchar = nc.dram_tensor("cchar", [NBC * CLEN, WLEN], I32, kind="ExternalInput")
    qchar = nc.dram_tensor("qchar", [NBC * QPAD, WLEN], I32, kind="ExternalInput")
    cword = nc.dram_tensor("cword", [NBC * CLEN, 1], I32, kind="ExternalInput")
    qword = nc.dram_tensor("qword", [NBC * QPAD, 1], I32, kind="ExternalInput")
    cemb = nc.dram_tensor("cemb", [CHAR_V, CHAR_D], F32, kind="ExternalInput")
    wemb = nc.dram_tensor("wemb", [WORD_V, WD], F32, kind="ExternalInput")
    convw = nc.dram_tensor("convw", [NPOS, P, CC], F32, kind="ExternalInput")
    convb = nc.dram_tensor("convb", [CC, 1], F32, kind="ExternalInput")
    hww = nc.dram_tensor("hww", [2, 2, 2, 2, 101, 100], F32, kind="ExternalInput")
    wih_ctx = nc.dram_tensor("wih_ctx", [2, 2, 101, 4 * H], F32, kind="ExternalInput")
    wih_m1 = nc.dram_tensor("wih_m1", [2, 8, 101, 4 * H], F32, kind="ExternalInput")
    wih_m2 = nc.dram_tensor("wih_m2", [2, 2, 101, 4 * H], F32, kind="ExternalInput")
    wih_out = nc.dram_tensor("wih_out", [2, 2, 101, 4 * H], F32, kind="ExternalInput")
    whh_all = nc.dram_tensor("whh_all", [4, 2, H, 4 * H], F32, kind="ExternalInput")
    attv = nc.dram_tensor("attv", [6, H, 1], F32, kind="ExternalInput")
    headw = nc.dram_tensor("headw", [2, 10, 101, 1], F32, kind="ExternalInput")
    onesd = nc.dram_tensor("onesd", [1, 4096], F32, kind="ExternalInput")
    out_d = nc.dram_tensor("out", [2, NBC, CLEN], F32, kind="ExternalOutput")

    with ExitStack() as ctx:
        tc = ctx.enter_context(tile.TileContext(nc))
        cons = ctx.enter_context(tc.tile_pool(name="cons", bufs=1))
        sbuf = ctx.enter_context(tc.tile_pool(name="sbuf", bufs=2))
        sb3 = ctx.enter_context(tc.tile_pool(name="sb3", bufs=3))

        ident = cons.tile([P, P], F32, name="ident")
        make_identity(nc, ident[:])
        onesc = cons.tile([1, P], F32, name="onesc")
        nc.sync.dma_start(out=onesc[:], in_=onesd[:, :P])
        onescol = cons.tile([P, 1], F32, name="onescol")
        nc.vector.memset(onescol[:], 1.0)

        convw_sb = cons.tile([P, NPOS, CC], F32, name="convw_sb")
        nc.sync.dma_start(out=convw_sb[:], in_=convw[:].rearrange("a p c -> p a c"))
        convb_sb = cons.tile([CC, 1], F32, name="convb_sb")
        nc.sync.dma_start(out=convb_sb[:], in_=convb[:])
        hww_sb = cons.tile([101, 2, 2, 2, 2, 100], F32, name="hww_sb")
        nc.sync.dma_start(out=hww_sb[:], in_=hww[:].rearrange("l g o k r c -> r l g o k c"))
        attv_sb = cons.tile([H, 6], F32, name="attv_sb")
        nc.sync.dma_start(out=attv_sb[:], in_=attv[:].rearrange("v h o -> h (v o)"))
        headw_sb = cons.tile([101, 2, 10], F32, name="headw_sb")
        nc.sync.dma_start(out=headw_sb[:], in_=headw[:].rearrange("e c r o -> r e (c o)"))

        def load_wih(pool, dram, nchunks, lname):
            out = []
            for d in range(2):
                tiles = []
                for ci in range(nchunks):
                    wt = pool.tile([101, 4 * H], F32, name=f"wi_{lname}_{d}_{ci}",
                                   tag=f"wi_{lname}_{d}_{ci}")
                    nc.sync.dma_start(out=wt[:], in_=dram[d, ci, :, :])
                    tiles.append(wt)
                out.append(tiles)
            return out

        def load_whh(pool, li, lname):
            pair = []
            for d in range(2):
                wt = pool.tile([H, 4 * H], F32, name=f"wh_{lname}_{d}", tag=f"wh_{lname}_{d}")
                nc.sync.dma_start(out=wt[:], in_=whh_all[li, d, :, :])
                pair.append(wt)
            return pair

        wih_sb = {"ctx": load_wih(cons, wih_ctx, 2, "ctx")}
        whh_sb = {"ctx": load_whh(cons, 0, "ctx")}

        # ---- phase A: embeddings + char conv ----
        pearly_cm = tc.tile_pool(name="pearly", bufs=1)
        pearly = pearly_cm.__enter__()
        xc0 = pearly.tile([H + 1, T, 8], F32, name="xc0")
        xc1 = pearly.tile([H, T, 8], F32, name="xc1")
        nc.sync.dma_start(out=xc0[H:H + 1, :, :],
                          in_=onesd[:].rearrange("o (t n) -> o t n", t=T))
        nc.vector.memset(xc0[:H, :, 4:], 0.0)
        nc.vector.memset(xc1[:, :, 4:], 0.0)

        with tc.tile_pool(name="psA", bufs=1, space="PSUM") as psA:
            def emb_tile(idx_dram, widx_dram, row0, out0, out1, src0, src1, tag):
                """One 128-token gather+conv tile. out0/out1: dest APs for char
                conv/word features; src0/src1: matching source views of the
                [100,128] results (token selectors)."""
                ix = sb3.tile([P, WLEN], I32, name=f"ix{tag}", tag="ix")
                nc.sync.dma_start(out=ix[:], in_=idx_dram[row0:row0 + P, :])
                wx = sb3.tile([P, 1], I32, name=f"wx{tag}", tag="wx")
                nc.sync.dma_start(out=wx[:], in_=widx_dram[row0:row0 + P, :])
                E = sb3.tile([P, WLEN, CHAR_D], F32, name=f"E{tag}", tag="E")
                for j in range(WLEN):
                    nc.gpsimd.indirect_dma_start(
                        out=E[:, j, :], out_offset=None, in_=cemb[:],
                        in_offset=bass.IndirectOffsetOnAxis(ap=ix[:, j:j + 1], axis=0))
                W = sb3.tile([P, WD], F32, name=f"W{tag}", tag="W")
                nc.gpsimd.indirect_dma_start(
                    out=W[:], out_offset=None, in_=wemb[:],
                    in_offset=bass.IndirectOffsetOnAxis(ap=wx[:, :1], axis=0))
                Et_ps = psA.tile([P, P], F32, space="PSUM", name=f"Etp{tag}", tag="Etp")
                nc.tensor.transpose(out=Et_ps[:], in_=E[:].rearrange("p a b -> p (a b)"),
                                    identity=ident[:])
                Et = sb3.tile([P, P], F32, name=f"Et{tag}", tag="Et")
                nc.vector.tensor_copy(out=Et[:], in_=Et_ps[:])
                Wt_ps = psA.tile([WD, P], F32, space="PSUM", name=f"Wtp{tag}", tag="Wtp")
                nc.tensor.transpose(out=Wt_ps[:], in_=W[:], identity=ident[:])
                nc.vector.tensor_copy(out=out1, in_=src1(Wt_ps))
                cv = psA.tile([CC, NPOS, P], F32, space="PSUM", name=f"cv{tag}", tag="cv")
                for tt in range(NPOS):
                    nc.tensor.matmul(out=cv[:, tt, :], lhsT=convw_sb[:, tt, :], rhs=Et[:],
                                     start=(tt % 4 == 0), stop=True, skip_group_check=True)
                mx = sb3.tile([CC, P], F32, name=f"mx{tag}", tag="mx")
                cv_view = bass.AP(tensor=cv[:].tensor, offset=cv[:].offset,
                                  ap=[list(cv[:].ap[0]), [1, P], [P, NPOS]])
                nc.vector.tensor_reduce(out=mx[:], in_=cv_view,
                                        axis=mybir.AxisListType.X, op=OP.max)
                nc.scalar.activation(out=out0, in_=src0(mx), func=AF.Relu,
                                     bias=convb_sb[:])

            def csel(t):  # full 128-token selector
                return lambda tl: tl[:CC, :] if t == 0 else tl[:WD, :]

            for w in range(16):
                b, t4 = divmod(w, 4)
                emb_tile(cchar, cword, 128 * w,
                         xc0[0:CC, t4 * 128:(t4 + 1) * 128, b],
                         xc1[0:WD, t4 * 128:(t4 + 1) * 128, b],
                         csel(0), csel(1), f"c{w}")
            for w in range(2):
                def qdst(xt, rows):
                    base = xt[0:rows, 0:1, 4 + 2 * w]
                    return bass.AP(tensor=base.tensor, offset=base.offset,
                                   ap=[list(base.ap[0]), [1, 2], [8, QLEN]])

                def qsrc(rows):
                    def f(tl):
                        a = tl[:rows, :]
                        return bass.AP(tensor=a.tensor, offset=a.offset,
                                       ap=[list(a.ap[0]), [QPAD, 2], [1, QLEN]])
                    return f
                emb_tile(qchar, qword, 128 * w, qdst(xc0, CC), qdst(xc1, WD),
                         qsrc(CC), qsrc(WD), f"q{w}")

        # ---- phase A2: highway ----
        xs = [xc0, xc1]
        with tc.tile_pool(name="psH", bufs=2, space="PSUM") as psH:
            nx0 = pearly.tile([H + 1, T, 8], F32, name="nx0")
            nx1 = pearly.tile([H, T, 8], F32, name="nx1")
            nc.sync.dma_start(out=nx0[H:H + 1, :, :],
                              in_=onesd[:].rearrange("o (t n) -> o t n", t=T))
            for layer in range(2):
                for nch in range(8):
                    t0 = nch * 64
                    for oc in range(2):
                        hp = psH.tile([H, 64, 8], F32, space="PSUM",
                                      name=f"hp{layer}{nch}{oc}", tag="hp")
                        gp = psH.tile([H, 64, 8], F32, space="PSUM",
                                      name=f"gp{layer}{nch}{oc}", tag="gp")
                        for kc in range(2):
                            rows = 101 if kc == 0 else 100
                            rhs = xs[kc][:rows, t0:t0 + 64, :]
                            nc.tensor.matmul(out=hp[:], lhsT=hww_sb[:rows, layer, 0, oc, kc, :],
                                             rhs=rhs, start=(kc == 0), stop=(kc == 1))
                            nc.tensor.matmul(out=gp[:], lhsT=hww_sb[:rows, layer, 1, oc, kc, :],
                                             rhs=rhs, start=(kc == 0), stop=(kc == 1))
                        hs = sb3.tile([H, 64, 8], F32, name=f"hs{layer}{nch}{oc}", tag="hs")
                        gs = sb3.tile([H, 64, 8], F32, name=f"gs{layer}{nch}{oc}", tag="gs")
                        nc.scalar.activation(out=hs[:], in_=hp[:], func=AF.Relu)
                        nc.scalar.activation(out=gs[:], in_=gp[:], func=AF.Sigmoid)
                        xo = xs[oc][0:H, t0:t0 + 64, :]
                        nxs = [nx0, nx1] if layer == 0 else [xc0, xc1]
                        dst = nxs[oc][0:H, t0:t0 + 64, :]
                        nc.vector.tensor_tensor(out=hs[:], in0=hs[:], in1=xo, op=OP.subtract)
                        nc.vector.tensor_tensor(out=hs[:], in0=gs[:], in1=hs[:], op=OP.mult)
                        nc.vector.tensor_tensor(out=dst, in0=hs[:], in1=xo, op=OP.add)
                xs = [nx0, nx1] if layer == 0 else [xc0, xc1]
        xf0, xf1 = xs

        # ---- phase B: ctx scan ----
        Hf = cons.tile([H + 1, T, 8], F32, name="Hf")
        Hb = cons.tile([H, T, 8], F32, name="Hb")
        nc.sync.dma_start(out=Hf[H:H + 1, :, :],
                          in_=onesd[:].rearrange("o (t n) -> o t n", t=T))
        with tc.tile_pool(name="psB", bufs=1, space="PSUM") as psB:
            emit_bilstm(nc, sbuf, psB, [(xf0, H + 1), (xf1, H)],
                        wih_sb["ctx"], whh_sb["ctx"], [Hf, Hb], 8, "ctx",
                        reset_step=T - QLEN, seg=16)

        pearly_cm.__exit__(None, None, None)
        plate = ctx.enter_context(tc.tile_pool(name="plate", bufs=1))
        # ---- phase C: attention ----
        c2qf = plate.tile([H, T, NBC], F32, name="c2qf")
        c2qb = plate.tile([H, T, NBC], F32, name="c2qb")
        ccf = plate.tile([H, T, NBC], F32, name="ccf")
        ccb = plate.tile([H, T, NBC], F32, name="ccb")
        g4f = plate.tile([H, T, NBC], F32, name="g4f")
        g4b = plate.tile([H, T, NBC], F32, name="g4b")
        with tc.tile_pool(name="psC", bufs=2, space="PSUM") as psC, \
             tc.tile_pool(name="sbC", bufs=2) as sbC:
            for b in range(NBC):
                cwf = sbC.tile([H, T], F32, name=f"cwf{b}", tag="cwf")
                cwb = sbC.tile([H, T], F32, name=f"cwb{b}", tag="cwb")
                nc.vector.tensor_scalar_mul(cwf[:], Hf[0:H, :, b], attv_sb[:, 0:1])
                nc.vector.tensor_scalar_mul(cwb[:], Hb[0:H, :, b], attv_sb[:, 1:2])
                qlp = psC.tile([1, QLEN], F32, space="PSUM", name=f"qlp{b}", tag="psml")
                nc.tensor.matmul(out=qlp[:], lhsT=attv_sb[:, 2:3],
                                 rhs=Hf[0:H, 0:QLEN, 4 + b], start=True, stop=False)
                nc.tensor.matmul(out=qlp[:], lhsT=attv_sb[:, 3:4],
                                 rhs=Hb[0:H, 0:QLEN, 4 + b], start=False, stop=True)
                qls = sbC.tile([1, QLEN], F32, name=f"qls{b}", tag="qls")
                nc.vector.tensor_copy(out=qls[:], in_=qlp[:])
                qt_ps = psC.tile([QLEN, H, 2], F32, space="PSUM", name=f"qt{b}", tag="pbig")
                nc.tensor.transpose(out=qt_ps[:, :, 0], in_=Hf[0:H, 0:QLEN, 4 + b],
                                    identity=ident[:H, :H])
                nc.tensor.transpose(out=qt_ps[:, :, 1], in_=Hb[0:H, 0:QLEN, 4 + b],
                                    identity=ident[:H, :H])
                qt = sbC.tile([QLEN, H, 2], F32, name=f"qts{b}", tag="qts")
                nc.vector.tensor_copy(out=qt[:], in_=qt_ps[:])
                vsb = sbC.tile([P, 4], F32, name=f"vsb{b}", tag="vsb")
                ctm = sbC.tile([P, D2, 4], F32, name=f"ctm{b}", tag="ctm")
                for ic in range(4):
                    isl = slice(ic * P, (ic + 1) * P)
                    sp = psC.tile([P, QLEN], F32, space="PSUM", name=f"sp{b}{ic}", tag="pbig")
                    nc.tensor.matmul(out=sp[:], lhsT=cwf[:, isl],
                                     rhs=Hf[0:H, 0:QLEN, 4 + b], start=True, stop=False)
                    nc.tensor.matmul(out=sp[:], lhsT=cwb[:, isl],
                                     rhs=Hb[0:H, 0:QLEN, 4 + b], start=False, stop=False)
                    nc.tensor.matmul(out=sp[:], lhsT=onesc[:], rhs=qls[:],
                                     start=False, stop=True)
                    rmax = sbC.tile([P, 1], F32, name=f"rmax{b}{ic}", tag="rmax")
                    nc.vector.tensor_reduce(out=rmax[:], in_=sp[:],
                                            axis=mybir.AxisListType.X, op=OP.max)
                    nmax = sbC.tile([P, 1], F32, name=f"nmax{b}{ic}", tag="nmax")
                    nc.vector.tensor_scalar_mul(nmax[:], rmax[:], -1.0)
                    esb = sbC.tile([P, QLEN], F32, name=f"esb{b}{ic}", tag="esb")
                    nc.scalar.activation(out=esb[:], in_=sp[:], func=AF.Exp, bias=nmax[:])
                    rsum = sbC.tile([P, 1], F32, name=f"rsum{b}{ic}", tag="rsum")
                    nc.vector.tensor_reduce(out=rsum[:], in_=esb[:],
                                            axis=mybir.AxisListType.X, op=OP.add)
                    rinv = sbC.tile([P, 1], F32, name=f"rinv{b}{ic}", tag="rinv")
                    nc.vector.reciprocal(out=rinv[:], in_=rsum[:])
                    nc.vector.tensor_scalar_mul(esb[:], esb[:], rinv[:])
                    at_ps = psC.tile([QLEN, P], F32, space="PSUM", name=f"at{b}{ic}", tag="pbig")
                    nc.tensor.transpose(out=at_ps[:], in_=esb[:], identity=ident[:])
                    ats = sbC.tile([QLEN, P], F32, name=f"ats{b}{ic}", tag="ats")
                    nc.vector.tensor_copy(out=ats[:], in_=at_ps[:])
                    c2p = psC.tile([H, P, 2], F32, space="PSUM", name=f"c2p{b}{ic}", tag="pbig")
                    nc.tensor.matmul(out=c2p[:, :, 0], lhsT=qt[:, :, 0], rhs=ats[:],
                                     start=True, stop=True)
                    nc.tensor.matmul(out=c2p[:, :, 1], lhsT=qt[:, :, 1], rhs=ats[:],
                                     start=True, stop=True)
                    nc.vector.tensor_copy(out=c2qf[:, isl, b], in_=c2p[:, :, 0])
                    nc.vector.tensor_copy(out=c2qb[:, isl, b], in_=c2p[:, :, 1])
                    vp = psC.tile([P, 1], F32, space="PSUM", name=f"vp{b}{ic}", tag="psml")
                    nc.tensor.matmul(out=vp[:], lhsT=Hf[0:H, isl, b], rhs=attv_sb[:, 4:5],
                                     start=True, stop=False)
                    nc.tensor.matmul(out=vp[:], lhsT=Hb[0:H, isl, b], rhs=attv_sb[:, 5:6],
                                     start=False, stop=True)
                    nc.vector.tensor_tensor(out=vsb[:, ic:ic + 1], in0=vp[:], in1=rmax[:],
                                            op=OP.add)
                    ctp = psC.tile([P, D2], F32, space="PSUM", name=f"ctp{b}{ic}", tag="pbig")
                    nc.tensor.transpose(out=ctp[:, 0:H], in_=Hf[0:H, isl, b],
                                        identity=ident[:H, :H])
                    nc.tensor.transpose(out=ctp[:, H:D2], in_=Hb[0:H, isl, b],
                                        identity=ident[:H, :H])
                    nc.vector.tensor_copy(out=ctm[:, :, ic], in_=ctp[:])
                m1 = sbC.tile([P, 1], F32, name=f"m1{b}", tag="m1")
                nc.vector.tensor_reduce(out=m1[:], in_=vsb[:],
                                        axis=mybir.AxisListType.X, op=OP.max)
                m1t_ps = psC.tile([1, P], F32, space="PSUM", name=f"m1t{b}", tag="psml")
                nc.tensor.transpose(out=m1t_ps[:], in_=m1[:], identity=ident[:])
                m1t = sbC.tile([1, P], F32, name=f"m1ts{b}", tag="m1ts")
                nc.vector.tensor_copy(out=m1t[:], in_=m1t_ps[:])
                gm = sbC.tile([1, 1], F32, name=f"gm{b}", tag="gm")
                nc.vector.tensor_reduce(out=gm[:], in_=m1t[:],
                                        axis=mybir.AxisListType.X, op=OP.max)
                nc.vector.tensor_scalar_mul(gm[:], gm[:], -1.0)
                gmb_ps = psC.tile([P, 1], F32, space="PSUM", name=f"gmb{b}", tag="psml")
                nc.tensor.matmul(out=gmb_ps[:], lhsT=onesc[:], rhs=gm[:], start=True, stop=True)
                gmb = sbC.tile([P, 1], F32, name=f"gmbs{b}", tag="gmbs")
                nc.vector.tensor_copy(out=gmb[:], in_=gmb_ps[:])
                ev = sbC.tile([P, 4], F32, name=f"ev{b}", tag="ev")
                nc.scalar.activation(out=ev[:], in_=vsb[:], func=AF.Exp, bias=gmb[:])
                es = sbC.tile([P, 1], F32, name=f"es{b}", tag="es")
                nc.vector.tensor_reduce(out=es[:], in_=ev[:],
                                        axis=mybir.AxisListType.X, op=OP.add)
                ssp = psC.tile([1, 1], F32, space="PSUM", name=f"ssp{b}", tag="psml")
                nc.tensor.matmul(out=ssp[:], lhsT=es[:], rhs=onescol[:], start=True, stop=True)
                ss = sbC.tile([1, 1], F32, name=f"ss{b}", tag="ss")
                nc.vector.tensor_copy(out=ss[:], in_=ssp[:])
                nc.vector.reciprocal(out=ss[:], in_=ss[:])
                sb_ps = psC.tile([P, 1], F32, space="PSUM", name=f"sbp{b}", tag="psml")
                nc.tensor.matmul(out=sb_ps[:], lhsT=onesc[:], rhs=ss[:], start=True, stop=True)
                sbv = sbC.tile([P, 1], F32, name=f"sbv{b}", tag="sbv")
                nc.vector.tensor_copy(out=sbv[:], in_=sb_ps[:])
                batt = sbC.tile([P, 4], F32, name=f"batt{b}", tag="batt")
                nc.vector.tensor_scalar_mul(batt[:], ev[:], sbv[:])
                q2p = psC.tile([1, D2], F32, space="PSUM", name=f"q2p{b}", tag="psml")
                for ic in range(4):
                    nc.tensor.matmul(out=q2p[:], lhsT=batt[:, ic:ic + 1], rhs=ctm[:, :, ic],
                                     start=(ic == 0), stop=(ic == 3))
                q2ps = sbC.tile([1, D2], F32, name=f"q2ps{b}", tag="q2psb")
                nc.vector.tensor_copy(out=q2ps[:], in_=q2p[:])
                q2tp = psC.tile([H, 2], F32, space="PSUM", name=f"q2tp{b}", tag="psml")
                nc.tensor.transpose(out=q2tp[:, 0:1], in_=q2ps[:, 0:H], identity=ident[:1, :1])
                nc.tensor.transpose(out=q2tp[:, 1:2], in_=q2ps[:, H:D2], identity=ident[:1, :1])
                q2s = sbC.tile([H, 2], F32, name=f"q2s{b}", tag="q2s")
                nc.vector.tensor_copy(out=q2s[:], in_=q2tp[:])
                nc.vector.tensor_tensor(out=ccf[:, :, b], in0=Hf[0:H, :, b],
                                        in1=c2qf[:, :, b], op=OP.mult)
                nc.vector.tensor_tensor(out=ccb[:, :, b], in0=Hb[0:H, :, b],
                                        in1=c2qb[:, :, b], op=OP.mult)
                nc.vector.tensor_scalar_mul(g4f[:, :, b], Hf[0:H, :, b], q2s[:, 0:1])
                nc.vector.tensor_scalar_mul(g4b[:, :, b], Hb[0:H, :, b], q2s[:, 1:2])

        # ---- phases D/E/F: mod1, mod2, out scans ----
        g_chunks = [(Hf, H + 1), (Hb, H), (c2qf, H), (c2qb, H),
                    (ccf, H), (ccb, H), (g4f, H), (g4b, H)]
        M1f = plate.tile([H + 1, T, NBC], F32, name="M1f")
        M1b = plate.tile([H, T, NBC], F32, name="M1b")
        nc.sync.dma_start(out=M1f[H:H + 1, :, :],
                          in_=onesd[:, :T * NBC].rearrange("o (t n) -> o t n", t=T))
        with tc.tile_pool(name="psD", bufs=1, space="PSUM") as psD, \
             tc.tile_pool(name="pw1", bufs=1) as pw1:
            emit_bilstm(nc, sbuf, psD, g_chunks, load_wih(pw1, wih_m1, 8, "m1"),
                        load_whh(pw1, 1, "m1"), [M1f, M1b], NBC, "m1", seg=16)
        M2f = plate.tile([H + 1, T, NBC], F32, name="M2f")
        M2b = plate.tile([H, T, NBC], F32, name="M2b")
        nc.sync.dma_start(out=M2f[H:H + 1, :, :],
                          in_=onesd[:, :T * NBC].rearrange("o (t n) -> o t n", t=T))
        with tc.tile_pool(name="psE", bufs=1, space="PSUM") as psE, \
             tc.tile_pool(name="pw2", bufs=1) as pw2:
            emit_bilstm(nc, sbuf, psE, [(M1f, H + 1), (M1b, H)],
                        load_wih(pw2, wih_m2, 2, "m2"), load_whh(pw2, 2, "m2"),
                        [M2f, M2b], NBC, "m2", seg=16)
        Of = plate.tile([H + 1, T, NBC], F32, name="Of")
        Ob = plate.tile([H, T, NBC], F32, name="Ob")
        nc.sync.dma_start(out=Of[H:H + 1, :, :],
                          in_=onesd[:, :T * NBC].rearrange("o (t n) -> o t n", t=T))
        with tc.tile_pool(name="psF", bufs=1, space="PSUM") as psF, \
             tc.tile_pool(name="pw3", bufs=1) as pw3:
            emit_bilstm(nc, sbuf, psF, [(M2f, H + 1), (M2b, H)],
                        load_wih(pw3, wih_out, 2, "out"), load_whh(pw3, 3, "out"),
                        [Of, Ob], NBC, "out", seg=16)

        # ---- phase G: heads ----
        with tc.tile_pool(name="psG", bufs=2, space="PSUM") as psG:
            for e, mpair in ((0, (M2f, M2b)), (1, (Of, Ob))):
                chunks = g_chunks + [(mpair[0], H + 1), (mpair[1], H)]
                for b in range(NBC):
                    hp = psG.tile([1, T], F32, space="PSUM", name=f"hd{e}{b}", tag="hd")
                    for ci, (tl, rows) in enumerate(chunks):
                        nc.tensor.matmul(out=hp[:], lhsT=headw_sb[:rows, e, ci:ci + 1],
                                         rhs=tl[:rows, :, b], start=(ci == 0),
                                         stop=(ci == 9))
                    hsb = sbuf.tile([1, T], F32, name=f"hsb{e}{b}", tag="hsb")
                    nc.vector.tensor_copy(out=hsb[:], in_=hp[:])
                    nc.gpsimd.dma_start(out=out_d[e, b, :].unsqueeze(0), in_=hsb[:])

    nc.finalize()
    return nc


_NC_CACHE = None


def _prep_gates(Wih, Whh, b):
    def reord(W):
        i, f, g, o = np.split(W, 4, axis=0)
        return np.concatenate([i, f, o, 2.0 * g], axis=0)
    return reord(Wih), reord(Whh), reord(b[:, None])[:, 0]


def _pack_lstm(p, chunk_rows):
    nch = len(chunk_rows)
    wih = np.zeros((2, nch, 101, 4 * H), np.float32)
    whh = np.zeros((2, H, 4 * H), np.float32)
    for d, sfx in enumerate(["f", "b"]):
        wi, wh, bb = _prep_gates(np.asarray(p[f"Wih_{sfx}"], np.float32),
                                 np.asarray(p[f"Whh_{sfx}"], np.float32),
                                 np.asarray(p[f"b_{sfx}"], np.float32))
        for ci, (s0, s1) in enumerate(chunk_rows):
            wih[d, ci, :s1 - s0, :] = wi[:, s0:s1].T
        wih[d, 0, 100, :] = bb
        whh[d] = wh.T
    return wih, whh


def kernel(c_char, q_char, c_word, q_word, char_emb_W, char_conv_W, char_conv_b,
           word_emb_W, highway, ctx_lstm, mod_lstm1, mod_lstm2, out_lstm, attn, heads):
    global _NC_CACHE
    from concourse.bass_utils import run_bass_kernel_spmd

    c_char = np.asarray(c_char); q_char = np.asarray(q_char)
    c_word = np.asarray(c_word); q_word = np.asarray(q_word)

    convw = np.zeros((NPOS, P, CC), np.float32)
    cw = np.asarray(char_conv_W, np.float32)  # [100, 8, 5]
    for tt in range(NPOS):
        for kk in range(CW):
            for ee in range(CHAR_D):
                convw[tt, 8 * tt + 8 * kk + ee, :] = cw[:, ee, kk]
    convb = np.asarray(char_conv_b, np.float32).reshape(CC, 1)

    hww = np.zeros((2, 2, 2, 2, 101, 100), np.float32)
    for l in range(2):
        for gi, (Wk, bk) in enumerate((("Wl", "bl"), ("Wg", "bg"))):
            Wm = np.asarray(highway[Wk], np.float32)[l]
            bv = np.asarray(highway[bk], np.float32)[l]
            for oc in range(2):
                for kc in range(2):
                    hww[l, gi, oc, kc, :100, :] = \
                        Wm[oc * 100:(oc + 1) * 100, kc * 100:(kc + 1) * 100].T
                hww[l, gi, oc, 0, 100, :] = bv[oc * 100:(oc + 1) * 100]

    wih_ctx, whh_ctx = _pack_lstm(ctx_lstm, [(0, 100), (100, 200)])
    wih_m1, whh_m1 = _pack_lstm(mod_lstm1, [(i * 100, (i + 1) * 100) for i in range(8)])
    wih_m2, whh_m2 = _pack_lstm(mod_lstm2, [(0, 100), (100, 200)])
    wih_out, whh_out = _pack_lstm(out_lstm, [(0, 100), (100, 200)])
    whh_all = np.stack([whh_ctx, whh_m1, whh_m2, whh_out], 0)

    attv = np.stack([np.asarray(attn["w_cq"], np.float32)[0:100],
                     np.asarray(attn["w_cq"], np.float32)[100:200],
                     np.asarray(attn["w_q"], np.float32)[0:100],
                     np.asarray(attn["w_q"], np.float32)[100:200],
                     np.asarray(attn["w_c"], np.float32)[0:100],
                     np.asarray(attn["w_c"], np.float32)[100:200]], 0)[:, :, None]

    headw = np.zeros((2, 10, 101, 1), np.float32)
    for e, (Wg_, bg_, Wm_, bm_) in enumerate(
            ((heads["Wsg"], heads["bsg"], heads["Wsm"], heads["bsm"]),
             (heads["Weg"], heads["beg"], heads["Wem"], heads["bem"]))):
        Wg_ = np.asarray(Wg_, np.float32); Wm_ = np.asarray(Wm_, np.float32)
        for ci in range(8):
            headw[e, ci, :100, 0] = Wg_[ci * 100:(ci + 1) * 100]
        headw[e, 0, 100, 0] = float(np.asarray(bg_)) + float(np.asarray(bm_))
        headw[e, 8, :100, 0] = Wm_[0:100]
        headw[e, 9, :100, 0] = Wm_[100:200]

    if _NC_CACHE is None:
        _NC_CACHE = build_kernel()
    nc = _NC_CACHE

    shared = {
        "cemb": np.asarray(char_emb_W, np.float32),
        "wemb": np.asarray(word_emb_W, np.float32),
        "convw": convw, "convb": convb, "hww": hww,
        "wih_ctx": wih_ctx, "wih_m1": wih_m1, "wih_m2": wih_m2, "wih_out": wih_out,
        "whh_all": whh_all, "attv": attv.astype(np.float32), "headw": headw,
        "onesd": np.ones((1, 4096), np.float32),
    }
    in_maps = []
    for core in range(8):
        sl = slice(core * NBC, (core + 1) * NBC)
        qc = np.zeros((NBC, QPAD, WLEN), np.int64)
        qc[:, :QLEN, :] = q_char[sl]
        qw = np.zeros((NBC, QPAD), np.int64)
        qw[:, :QLEN] = q_word[sl]
        m = dict(shared)
        m["cchar"] = np.ascontiguousarray(c_char[sl].reshape(NBC * CLEN, WLEN)).astype(np.int32)
        m["qchar"] = qc.reshape(NBC * QPAD, WLEN).astype(np.int32)
        m["cword"] = np.ascontiguousarray(c_word[sl].reshape(NBC * CLEN, 1)).astype(np.int32)
        m["qword"] = qw.reshape(NBC * QPAD, 1).astype(np.int32)
        in_maps.append(m)

    res = run_bass_kernel_spmd(nc, in_maps, core_ids=list(range(8)))
    outs = [r["out"] for r in res.results]
    full = np.concatenate(outs, axis=1)
    return np.stack([full[0], full[1]], 0).astype(np.float32)
